# revision 23
# baseline (speedup 1.0000x reference)
"""Causal GQA self-attention kernel for Trainium2 (8 NeuronCores).

Sharding: 8 cores = batch (2) x kv-head-group (4). Each core computes, for
its (batch b, kv group g): the Q projection for the group's 4 query heads,
K/V projections for its kv head, causal flash attention for those heads,
and the partial output projection (rows of Wo for its heads). The host
sums the 4 partial outputs per batch element.

Schedule (v8): fully software-pipelined phases.
  - weights are host-packed partition-major so DMA descriptors are fat
    (>=1KB contiguous runs -> full 360GB/s model rate); startup DMAs are
    split so the first K-projection matmul issues ~3us in.
  - V projection is computed directly in [token, d] layout (x strips as
    the stationary operand, 128-token sub-blocks), eliminating the PE
    transpose + extra copies of v7.
  - window structure: proj chunk 0 | proj 1 + attn 0 | proj 2 + attn 1 +
    2 out-tiles | projQ 3 + attn 2 + projKV 3 + 4 out-tiles | attn 3 +
    6 out-tiles | 4 out-tiles tail. Attention heads keep the v7 deep
    pipeline (S runs ahead in 2 PSUM banks, Y matmuls deferred la_look+2
    strips behind their exp, leftovers ride into the next unit's stream,
    softmax epilogue deferred into the next head's strips).
  - PSUM budget: proj 2 + S 2 + Y 2 + (out-proj|epilogue) 2 = 8 banks.
  - engine balance: exp on ACT only; causal-triangle mask and most
    projection copies on Pool; la accumulation (bf16 2x), reciprocal and
    yT scaling on DVE; out-proj PSUM->SBUF copies rotate engines per
    window so ACT stays exp-only where exp saturates it.
"""
import math

import numpy as np
import ml_dtypes

import concourse.mybir as mybir
from concourse import bacc
from concourse.tile import TileContext
from concourse.bass_utils import run_bass_kernel_spmd

F32 = mybir.dt.float32
BF16 = mybir.dt.bfloat16

E = 2048          # embedding dim
T = 2048          # sequence length
D = 128           # head dim
G = 4             # query heads per core (= GQA group size)
C = G * D         # 512 projected columns per core
KT = E // 128     # 16 contraction strips
NTCH = T // 512   # 4 t-chunks
SCALE = 1.0 / math.sqrt(D)


def build_attn(repeat: int = 1, sps_bufs: int = 2, o_bufs: int = 2,
               p_bufs: int = 12, y_bufs: int = 2, la_look: int = 4,
               ot_bufs: int = 4, psp_bufs: int = 2, warmup: int = 8,
               tail_split: int = 4, tail_split14: int = 1):
    nc = bacc.Bacc()
    xT = nc.dram_tensor("xT", [E, T], BF16, kind="ExternalInput")
    wq = nc.dram_tensor("wq", [128, G * KT * 128], BF16, kind="ExternalInput")
    wk = nc.dram_tensor("wk", [128, KT * D], BF16, kind="ExternalInput")
    wv = nc.dram_tensor("wv", [128, KT * D], BF16, kind="ExternalInput")
    wo = nc.dram_tensor("wo", [128, G * E], BF16, kind="ExternalInput")
    tri = nc.dram_tensor("tri", [128, 128], BF16, kind="ExternalInput")
    out = nc.dram_tensor("out", [T, E], BF16, kind="ExternalOutput")

    xT3 = xT.rearrange("(ko p) t -> p ko t", p=128)
    wq4 = wq.rearrange("p (j k c) -> p j k c", j=G, k=KT)
    wk3 = wk.rearrange("p (k d) -> p k d", k=KT)
    wv3 = wv.rearrange("p (k d) -> p k d", k=KT)
    wo3 = wo.rearrange("p (g e) -> p g e", g=G)

    with TileContext(nc) as tc:
        with tc.tile_pool(name="persist", bufs=1) as persist, \
             tc.tile_pool(name="xp", bufs=2) as xpool, \
             tc.tile_pool(name="sb", bufs=1) as sb23, \
             tc.tile_pool(name="ps", bufs=1, space="PSUM") as ps23:
            qT_sb = persist.tile([128, G, T], BF16)
            kT_sb = persist.tile([128, T], BF16)
            v_sb = persist.tile([128, 16, 128], BF16)
            yT_sb = persist.tile([128, G, T], BF16)
            tri_sb = persist.tile([128, 128], BF16)
            scratch = persist.tile([128, 128], F32)
            ones_b = persist.tile([128, 128], BF16)
            wq_sb = persist.tile([128, G, KT, 128], BF16)
            wk_sb = persist.tile([128, KT, D], BF16)
            wv_sb = persist.tile([128, KT, D], BF16)
            wo_sb = persist.tile([128, G, E], BF16)
            nc.vector.memset(scratch[:], 1.0)
            nc.scalar.copy(ones_b[:], scratch[:])

            def cp(eng, dst, src):
                if eng == "a":
                    nc.scalar.copy(dst, src)
                elif eng == "v":
                    nc.vector.tensor_copy(dst, src)
                else:
                    nc.gpsimd.tensor_copy(dst, src)

            def body():
                dma = nc.sync.dma_start
                xblks = [None] * NTCH

                def xpiece(c, k0, k1):
                    t0 = c * 512
                    dma(xblks[c][:, k0:k1, :], xT3[:, k0:k1, t0:t0 + 512])

                # ---- startup DMAs (order = SP queue order) ----
                xblks[0] = xpool.tile([128, KT, 512], BF16, tag="xb",
                                      name="xb0")
                dma(wk_sb[:, 0:4, :], wk3[:, 0:4, :])
                xpiece(0, 0, 1)
                dma(wv_sb[:, 0:4, :], wv3[:, 0:4, :])
                xpiece(0, 1, 4)
                dma(wk_sb[:, 4:KT, :], wk3[:, 4:KT, :])
                dma(wv_sb[:, 4:KT, :], wv3[:, 4:KT, :])
                xpiece(0, 4, 8)
                xpiece(0, 8, 12)
                xpiece(0, 12, 16)
                dma(tri_sb[:], tri[:])
                for j in range(G):
                    dma(wq_sb[:, j], wq4[:, j])
                xblks[1] = xpool.tile([128, KT, 512], BF16, tag="xb",
                                      name="xb1")
                xpiece(1, 0, 4)
                xpiece(1, 4, 16)
                dma(wo_sb[:], wo3)

                # ---- projection units ----
                def proj_kv(c, engk, engv):
                    t0 = c * 512
                    psk = ps23.tile([128, 512], F32, tag="psp", bufs=psp_bufs)
                    psv = ps23.tile([128, 4, 128], F32, tag="psp",
                                    bufs=psp_bufs)
                    for k in range(KT):
                        nc.tensor.matmul(psk[:], (wk_sb[:, k, :]),
                                         (xblks[c][:, k, :]),
                                         start=(k == 0), stop=(k == KT - 1))
                    cp(engk, kT_sb[:, t0:t0 + 512], psk[:])
                    for i in range(4):
                        for k in range(KT):
                            nc.tensor.matmul(
                                psv[:, i, :],
                                (xblks[c][:, k, i * 128:(i + 1) * 128]),
                                (wv_sb[:, k, :]),
                                start=(k == 0), stop=(k == KT - 1))
                    cp(engv, v_sb[:, c * 4:(c + 1) * 4, :], psv[:])

                def proj_q(c, j, eng):
                    t0 = c * 512
                    psq = ps23.tile([128, 512], F32, tag="psp", bufs=psp_bufs)
                    for k in range(KT):
                        nc.tensor.matmul(psq[:], (wq_sb[:, j, k, :]),
                                         (xblks[c][:, k, :]),
                                         start=(k == 0), stop=(k == KT - 1))
                    cp(eng, qT_sb[:, j, t0:t0 + 512], psq[:])

                # ---- attention machinery (v7 deep pipeline) ----
                pending = [None]

                def flush():
                    if pending[0] is not None:
                        pending[0]()
                        pending[0] = None

                ydq = []  # deferred Y matmuls, carried across units

                def emit_y(item):
                    s, v0, p, yps, n = item
                    nc.tensor.matmul(
                        yps[:, v0:], (v_sb[:, s, :]), (p[:, v0:512]),
                        start=(s == 0), stop=(s == n - 1))

                def attn_head(q, h):
                    tq0 = q * 512
                    n = 4 * (q + 1)
                    flush_gi = min(3, n - 1)
                    la = sb23.tile([128, 512], BF16, tag="la", bufs=2)
                    yps = ps23.tile([128, 512], F32, tag="y", bufs=y_bufs)
                    for s in range(n):
                        o = s - (n - 4)
                        v0 = max(0, 128 * o)
                        cs = v0
                        sps = ps23.tile([128, 512], F32, tag="s",
                                        bufs=sps_bufs)
                        nc.tensor.matmul(
                            sps[:, 0:512 - cs],
                            (kT_sb[:, s * 128:(s + 1) * 128]),
                            (qT_sb[:, h, tq0 + cs:tq0 + 512]),
                            start=True, stop=True)
                        if s == 0:
                            while len(ydq) > la_look:
                                emit_y(ydq.pop(0))
                        elif s == flush_gi:
                            flush()
                        p = sb23.tile([128, 512], BF16, tag="p",
                                      bufs=p_bufs)
                        nc.scalar.activation(
                            p[:, v0:512], sps[:, v0 - cs:512 - cs],
                            mybir.ActivationFunctionType.Exp,
                            scale=SCALE)
                        if o >= 0:
                            nc.gpsimd.tensor_mul(
                                p[:, v0:v0 + 128], p[:, v0:v0 + 128],
                                tri_sb[:])
                        if s == 0:
                            nc.vector.tensor_copy(la[:], p[:])
                        else:
                            nc.vector.tensor_add(
                                la[:, v0:], la[:, v0:], p[:, v0:])
                        ydq.append((s, v0, p, yps, n))
                        while len(ydq) > la_look + 2:
                            emit_y(ydq.pop(0))
                    while len(ydq) > la_look:
                        emit_y(ydq.pop(0))

                    def make_epi(h=h, tq0=tq0, la=la, yps=yps,
                                 drain=list(ydq)):
                        def epi():
                            for item in drain:
                                if item in ydq:
                                    ydq.remove(item)
                                    emit_y(item)
                            rps = ps23.tile([128, 512], F32, tag="o",
                                            bufs=o_bufs)
                            nc.tensor.matmul(rps[:], (ones_b[:]), (la[:]),
                                             start=True, stop=True)
                            rinv = sb23.tile([128, 512], F32,
                                             tag="rinv", bufs=2)
                            nc.vector.reciprocal(rinv[:], rps[:])
                            for pc in range(4):
                                c0 = pc * 128
                                nc.vector.tensor_mul(
                                    yT_sb[:, h, tq0 + c0:tq0 + c0 + 128],
                                    yps[:, c0:c0 + 128],
                                    rinv[:, c0:c0 + 128])
                        return epi

                    pending[0] = make_epi()

                def emit_tile(tt, cp_engines, dma_split=1, dma_fn=None,
                              tags="oooo"):
                    odma = dma_fn or dma
                    ot = sb23.tile([128, E], BF16, tag="ot", bufs=ot_bufs)
                    per = 4 // dma_split
                    tagbufs = {"o": o_bufs, "s": sps_bufs, "y": y_bufs}
                    for ech in range(4):
                        e0 = ech * 512
                        tg = tags[ech]
                        pso = ps23.tile([128, 512], F32, tag=tg,
                                        bufs=tagbufs[tg])
                        for hh in range(G):
                            nc.tensor.matmul(
                                pso[:],
                                (yT_sb[:, hh, tt * 128:(tt + 1) * 128]),
                                (wo_sb[:, hh, e0:e0 + 512]),
                                start=(hh == 0), stop=(hh == G - 1))
                        cp(cp_engines[ech], ot[:, e0:e0 + 512], pso[:])
                        if (ech + 1) % per == 0:
                            d0 = (ech + 1 - per) * 512
                            odma(out[tt * 128:(tt + 1) * 128,
                                     d0:e0 + 512],
                                 ot[:, d0:e0 + 512])

                # ---- master schedule ----
                # PE warmup: dummy matmuls while the first x/w DMAs land,
                # so the PE clock is ramped when real projections start
                for w in range(warmup):
                    wps = ps23.tile([128, 512], F32, tag="s", bufs=sps_bufs)
                    nc.tensor.matmul(wps[:, 0:128], (scratch[:]),
                                     (scratch[:]), start=True, stop=True)

                # window 0: chunk-0 K+V interleaved strip-by-strip so
                # consumption matches the x DMA feed rate; each V
                # sub-block gets its own (attention-idle) PSUM bank
                psk0 = ps23.tile([128, 512], F32, tag="psp", bufs=psp_bufs)
                vtags = [("s", sps_bufs), ("s", sps_bufs),
                         ("y", y_bufs), ("y", y_bufs)]
                psv0 = [ps23.tile([128, 512], F32, tag=t, bufs=b,
                                  name=f"psv0_{i}")
                        for i, (t, b) in enumerate(vtags)]
                for k in range(KT):
                    nc.tensor.matmul(psk0[:], (wk_sb[:, k, :]),
                                     (xblks[0][:, k, :]),
                                     start=(k == 0), stop=(k == KT - 1))
                    for i in range(4):
                        nc.tensor.matmul(
                            psv0[i][:, 0:128],
                            (xblks[0][:, k, i * 128:(i + 1) * 128]),
                            (wv_sb[:, k, :]),
                            start=(k == 0), stop=(k == KT - 1))
                cp("v", kT_sb[:, 0:512], psk0[:])
                for i in range(4):
                    cp("av"[i % 2], v_sb[:, i, :], psv0[i][:, 0:128])
                for j in range(G):
                    proj_q(0, j, "av"[j % 2])

                # window 1: proj chunk 1 + attention chunk 0
                proj_kv(1, "v", "a")
                for j in range(G):
                    attn_head(0, j)
                    proj_q(1, j, "av"[j % 2])
                # x chunk 2 (xpool buf0 free once chunk-0 reads are done)
                xblks[2] = xpool.tile([128, KT, 512], BF16, tag="xb",
                                      name="xb2")
                xpiece(2, 0, 4)
                xpiece(2, 4, 16)

                # window 2: proj chunk 2 + attention chunk 1 + 2 out tiles
                proj_kv(2, "v", "a")
                for j in range(G):
                    attn_head(1, j)
                    proj_q(2, j, "av"[j % 2])
                xblks[3] = xpool.tile([128, KT, 512], BF16, tag="xb",
                                      name="xb3")
                xpiece(3, 0, 4)
                xpiece(3, 4, 16)
                emit_tile(0, "aava")
                emit_tile(1, "vava")

                # window 3: proj-Q chunk 3 + attention chunk 2 +
                #           proj-K/V chunk 3 + 4 out tiles
                for j in range(G):
                    proj_q(3, j, "v")
                    attn_head(2, j)
                proj_kv(3, "v", "v")
                emit_tile(2, "avva")
                emit_tile(3, "vava")
                emit_tile(4, "avva")
                emit_tile(5, "vava")

                # window 4: attention chunk 3 + 6 out tiles
                attn_head(3, 0)
                emit_tile(6, "vvvv")
                attn_head(3, 1)
                emit_tile(7, "vvvv")
                attn_head(3, 2)
                emit_tile(8, "vvvv")
                attn_head(3, 3)
                # one tile of PE cover for the last exps, drain the Ys,
                # then two more tiles BEFORE the flush: the epilogue's
                # rps matmul waits on the last la adds (DVE), and PE is
                # in-order — anything queued behind it would stall too
                emit_tile(9, "vvvv", tags="osos")
                for item in list(ydq):
                    emit_y(item)
                ydq.clear()
                emit_tile(10, "vvvv", tags="osos")
                emit_tile(11, "vvvv", tags="osos")
                flush()

                # tail: chunk-3 out tiles
                emit_tile(12, "avva", tags="osyo")
                emit_tile(13, "avav", tags="syos")
                emit_tile(14, "avav", dma_split=tail_split14, tags="yosy")
                emit_tile(15, "aava", dma_split=tail_split, tags="osyo")

            if repeat == 1:
                body()
            else:
                for _rep in range(repeat):
                    if _rep:
                        tc.strict_bb_all_engine_barrier()
                    body()

    nc.compile()
    return nc


def _make_mask():
    r = np.arange(128)[:, None]
    c = np.arange(128)[None, :]
    return (c >= r).astype(ml_dtypes.bfloat16)


def _pack_pmajor(w, ncols):
    """[KT*128, ncols] -> [128, KT*ncols] partition-major contiguous."""
    return np.ascontiguousarray(
        w.reshape(KT, 128, ncols).transpose(1, 0, 2).reshape(128, KT * ncols))


def make_in_maps(x, Wq, Wk, Wv, Wo):
    """Host-side shard + bf16 cast + weight packing: one map per core."""
    x = np.asarray(x, dtype=np.float32)
    B = x.shape[0]
    assert x.shape == (B, T, E)
    xTh = np.ascontiguousarray(np.transpose(x, (0, 2, 1))).astype(
        ml_dtypes.bfloat16)
    Wqb = np.asarray(Wq, np.float32).astype(ml_dtypes.bfloat16)
    Wkb = np.asarray(Wk, np.float32).astype(ml_dtypes.bfloat16)
    Wvb = np.asarray(Wv, np.float32).astype(ml_dtypes.bfloat16)
    Wob = np.asarray(Wo, np.float32).astype(ml_dtypes.bfloat16)
    mask_np = _make_mask()
    in_maps = []
    for core in range(8):
        b, g = divmod(core, 4)
        b = b % B
        # wq: [E, C] -> [128, KT, G, 128] -> column-block-major [128, G*KT*128]
        wq_slice = Wqb[:, g * C:(g + 1) * C].reshape(KT, 128, G, 128)
        wq_p = np.ascontiguousarray(
            wq_slice.transpose(1, 2, 0, 3).reshape(128, G * KT * 128))
        # wo: [C, E] -> [128, G*E]
        wo_slice = Wob[g * C:(g + 1) * C, :].reshape(G, 128, E)
        wo_p = np.ascontiguousarray(
            wo_slice.transpose(1, 0, 2).reshape(128, G * E))
        in_maps.append({
            "xT": xTh[b],
            "wq": wq_p,
            "wk": _pack_pmajor(Wkb[:, g * D:(g + 1) * D], D),
            "wv": _pack_pmajor(Wvb[:, g * D:(g + 1) * D], D),
            "wo": wo_p,
            "tri": mask_np,
        })
    return in_maps


_NC = None


def kernel(x, Wq, Wk, Wv, Wo):
    global _NC
    if _NC is None:
        _NC = build_attn(repeat=1)
    nc = _NC

    B = np.asarray(x).shape[0]
    in_maps = make_in_maps(x, Wq, Wk, Wv, Wo)
    res = run_bass_kernel_spmd(nc, in_maps, list(range(8))).results
    outp = np.empty((B, T, E), dtype=np.float32)
    for b in range(B):
        acc = res[4 * b]["out"].astype(np.float64)
        for g in range(1, 4):
            acc += res[4 * b + g]["out"]
        outp[b] = acc.astype(np.float32)
    return outp
